# revision 1
# baseline (speedup 1.0000x reference)
"""Trainium2 Bass kernel for causal multi-head attention + output projection.

Problem (hardcoded): x[4, 2048, 1024] fp32, 16 heads, head_dim 64, causal,
torch-Linear convention (y = x @ W.T), output projection with bias.

Sharding over 8 NeuronCores: batch (4) x head-group (2 groups of 8 heads).
Core c = (b, g): computes q/k/v for heads [8g, 8g+8) of batch b, causal
attention in the S^T layout (keys on partitions, queries on free dim), a
partial output projection over its own 512 O-dims for all 2048 queries, and
a chunked ReduceScatter(add) over the core pair that leaves each core with
the final y for one query half (rank order decides which -> the program is
identical on all cores; the host just concatenates the halves).

Performance structure (single emission stream, software-pipelined):
  - attention runs per head-pair p at query-block granularity (4 blocks of
    512 queries).  The two heads of a pair share one 2-bank PSUM tile for
    S^T (head at col 0 / 512 -> different banks), so the two row-tiled
    (64x128) S matmuls co-execute on PE tiles (0,0)/(64,0), and the softmax
    exp for both heads is ONE ACT instruction on a strided [128, 2, n]
    view.  ACT (exp) is the critical engine (~158us); everything else hides
    behind it.
  - QKV matmul groups for pair p+1 are emitted between attention j-batches
    of pair p so the PE fills the exp-wait gaps.
  - softmax denominators ride as a ones-column in V (row 64 of the O psum);
    normalization = reciprocal + partition-broadcast via a small DRAM
    bounce, applied in SBUF off the critical path.

All matmul operands are bf16 (~0.3% rel err, same PE throughput as fp32r,
half the DMA/SBUF/collective traffic).  PSUM accumulation is fp32.
"""
import sys
import types
from contextlib import ExitStack

import numpy as np

import concourse.bass as bass
import concourse.mybir as mybir
import concourse.tile as tile
from concourse import bacc, bass_utils

F32 = mybir.dt.float32
BF16 = mybir.dt.bfloat16
AF = mybir.ActivationFunctionType
OP = mybir.AluOpType

import os as _os
_DEBUG_DUMP = bool(int(_os.environ.get("ATTN_DEBUG_DUMP", "0")))
_AV_SPLIT = bool(int(_os.environ.get("ATTN_AV_SPLIT", "1")))
_JBATCH = int(_os.environ.get("ATTN_JBATCH", "2"))

B, T, D = 4, 2048, 1024
HG = 8           # heads per core
NP = 4           # head pairs per core
QB = 512         # query block
NQB = T // QB    # 4 query blocks
QH = 1024        # query half (per-core output rows)
N_CORES = 8
SCALE = 1.0 / 8.0
MODE = "rs"  # harness compat


# ---------------------------------------------------------------------------
# environment glue
# ---------------------------------------------------------------------------

def _install_ntff_hook():
    if 'antenv.axon_hooks' in sys.modules:
        return
    try:
        from trn_agent_boot.trn_boot import _ntff_profile_via_ctypes
        hook = _ntff_profile_via_ctypes('/opt/axon/libaxon_pjrt.so')
    except Exception:
        hook = None
    mod = types.ModuleType('antenv.axon_hooks')
    mod.get_axon_ntff_profile_hook = lambda: hook
    mod.set_axon_ntff_profile_hook = lambda h: None
    sys.modules['antenv.axon_hooks'] = mod


def _run_spmd(nc, in_maps, trace=False):
    from concourse.bass_interp import get_hw_module
    bass_utils.upload_artifacts = lambda tmpdir: tmpdir
    if trace:
        _install_ntff_hook()
    old_m = nc.m
    nc.m = get_hw_module(nc.m)
    try:
        return bass_utils.run_bass_kernel_spmd(
            nc, in_maps, core_ids=list(range(N_CORES)),
            trace=trace, trace_cores=[0] if trace else None,
        )
    finally:
        nc.m = old_m


# ---------------------------------------------------------------------------
# kernel program
# ---------------------------------------------------------------------------

def build_nc():
    nc = bacc.Bacc("TRN2", target_bir_lowering=False, debug=False,
                   enable_asserts=False, num_devices=N_CORES)
    xT = nc.dram_tensor("xT", [D, T], BF16, kind="ExternalInput").ap()
    wqT = nc.dram_tensor("wqT", [D, 512], BF16, kind="ExternalInput").ap()
    wkT = nc.dram_tensor("wkT", [D, 512], BF16, kind="ExternalInput").ap()
    wvT = nc.dram_tensor("wvT", [D, 512], BF16, kind="ExternalInput").ap()
    wpT = nc.dram_tensor("wpT", [512, D], BF16, kind="ExternalInput").ap()
    bias = nc.dram_tensor("bias", [1, D], F32, kind="ExternalInput").ap()
    mask = nc.dram_tensor("mask", [128, 128], BF16, kind="ExternalInput").ap()
    vone = nc.dram_tensor("vone", [128, 32], BF16, kind="ExternalInput").ap()
    snum = nc.dram_tensor("snum", [16, 2 * QB], F32).ap()
    srecd = nc.dram_tensor("srecd", [16, 2 * QB], F32).ap()
    # y_part[c, half] = partial y rows [1024*half + 256*c, +256) so each
    # ReduceScatter chunk (one c) is contiguous
    y_part = nc.dram_tensor("y_part", [4, 2, 256, D], BF16).ap()
    yred = nc.dram_tensor("yred", [QH, D], BF16).ap()
    yout = nc.dram_tensor("yout", [QH, D], BF16, kind="ExternalOutput").ap()
    dbg = None
    if _DEBUG_DUMP:
        dbg = nc.dram_tensor("dbg", [128, NP, T], BF16,
                             kind="ExternalOutput").ap()

    with tile.TileContext(nc) as tc, ExitStack() as ctx:
        per = ctx.enter_context(tc.tile_pool(name="per", bufs=1))

        mask_sb = per.tile([128, 128], BF16, tag="mask")
        nc.sync.dma_start(mask_sb[:], mask[:])
        qT_sb = per.tile([128, NP, T], BF16, tag="qT")
        kT_sb = per.tile([128, NP, T], BF16, tag="kT")
        o_keep = per.tile([128, NP, T], BF16, tag="okeep")
        bias_bc = per.tile([128, D], F32, tag="bbc")
        wp_sb = per.tile([128, NP, D], BF16, tag="wp")

        wpool = ctx.enter_context(tc.tile_pool(name="wpool", bufs=1))
        xpool = ctx.enter_context(tc.tile_pool(name="xpool", bufs=1))

        xT_r = xT.rearrange("(ko ki) t -> ki ko t", ki=128)

        def load_w(wT, nm):
            parts = []
            wT_r = wT.rearrange("(ko ki) n -> ki ko n", ki=128)
            for kk in range(8):
                t = wpool.tile([128, 512], BF16, tag=f"w{nm}{kk}",
                               name=f"w{nm}{kk}")
                nc.sync.dma_start(t[:], wT_r[:, kk])
                parts.append(t)
            return parts

        # load order = first-consumption order: wk, x(th0), wq, wv, rest of
        # x, then the projection-only tensors
        wk_sb = load_w(wkT, "k")
        xh = [[None] * NQB for _ in range(8)]

        def load_x(th):
            for kk in range(8):
                t = xpool.tile([128, QB], BF16, tag=f"x{kk}_{th}",
                               name=f"x{kk}_{th}")
                nc.sync.dma_start(t[:], xT_r[:, kk, th * QB:(th + 1) * QB])
                xh[kk][th] = t

        load_x(0)
        wq_sb = load_w(wqT, "q")
        wv_sb = load_w(wvT, "v")
        for th in range(1, NQB):
            load_x(th)
        nc.sync.dma_start(bias_bc[:], bias[0][None, :].broadcast_to([128, D]))
        nc.sync.dma_start(wp_sb[:],
                          wpT.rearrange("(ko ki) n -> ki ko n", ki=128))

        with ExitStack() as attn_ctx:
            vpool = attn_ctx.enter_context(tc.tile_pool(name="vpool", bufs=2))
            qkps = attn_ctx.enter_context(
                tc.tile_pool(name="qkps", bufs=2, space="PSUM"))
            sps = attn_ctx.enter_context(
                tc.tile_pool(name="sps", bufs=2, space="PSUM"))
            ops = attn_ctx.enter_context(
                tc.tile_pool(name="ops", bufs=2, space="PSUM"))
            epool = attn_ctx.enter_context(tc.tile_pool(name="epool", bufs=3))
            npool = attn_ctx.enter_context(tc.tile_pool(name="npool", bufs=4))
            ypool = attn_ctx.enter_context(tc.tile_pool(name="ypool", bufs=3))

            v_tiles = {}

            # ----------------------------------------------------------
            # background QKV emission chain
            # ----------------------------------------------------------
            def qkv_chain():
                """List of (emit_fn, marker); marker=(p, th) means that
                after this group, pair p's q/k/v for tokens/keys up to
                512*(th+1) are fully emitted."""
                chain = []
                for p in range(NP):
                    def ones_dma(p=p):
                        v_sb = vpool.tile([128, 16, 2, 65], BF16, tag="v",
                                          name=f"v{p}")
                        v_tiles[p] = v_sb
                        nc.sync.dma_start(
                            v_sb[:, :, :, 64],
                            vone.rearrange("q (a b) -> q a b", a=16))
                    chain.append((ones_dma, None))
                    for th in range(NQB):
                        for wsb, dst in ((wk_sb, kT_sb), (wq_sb, qT_sb)):
                            box = {}

                            def fill(half, box=box, wsb=wsb, th=th, p=p):
                                if half == 0:
                                    box["pt"] = qkps.tile([128, QB], F32,
                                                          tag="pt", name="pt")
                                pt = box["pt"]
                                for kk in range(4 * half, 4 * half + 4):
                                    nc.tensor.matmul(
                                        pt[:],
                                        lhsT=wsb[kk][:, p * 128:(p + 1) * 128],
                                        rhs=xh[kk][th][:],
                                        start=(kk == 0), stop=(kk == 7))

                            def evict(box=box, dst=dst, th=th, p=p):
                                nc.scalar.copy(
                                    dst[:, p, th * QB:(th + 1) * QB],
                                    box["pt"][:])
                            chain.append((lambda f=fill: f(0), None))
                            chain.append((lambda f=fill: f(1), None))
                            chain.append((evict, None))
                        # V for key blocks 4*th .. 4*th+3
                        box = {}

                        def vfill(sub, box=box, th=th, p=p):
                            if sub == 0:
                                box["pt"] = qkps.tile([128, QB], F32,
                                                      tag="pt", name="pt")
                            pt = box["pt"]
                            for kk in range(8):
                                nc.tensor.matmul(
                                    pt[:, sub * 128:(sub + 1) * 128],
                                    lhsT=xh[kk][th][:,
                                                    sub * 128:(sub + 1) * 128],
                                    rhs=wv_sb[kk][:, p * 128:(p + 1) * 128],
                                    start=(kk == 0), stop=(kk == 7))

                        def vevict(box=box, th=th, p=p):
                            nc.scalar.copy(
                                v_tiles[p][:, 4 * th:4 * th + 4, :, 0:64],
                                box["pt"][:].rearrange(
                                    "q (m h d) -> q m h d", m=4, h=2))
                        for sub in range(4):
                            chain.append((lambda f=vfill, s=sub: f(s), None))
                        chain.append((vevict, (p, th)))
                return chain

            chain = qkv_chain()
            pos = [0]
            emitted = {}

            def emit_next():
                if pos[0] >= len(chain):
                    return False
                fn, marker = chain[pos[0]]
                pos[0] += 1
                fn()
                if marker is not None:
                    emitted[marker[0]] = marker[1]
                return True

            def drain_until(p, th):
                while emitted.get(p, -1) < th:
                    if not emit_next():
                        raise RuntimeError("qkv chain exhausted early")

            def feeder(k):
                for _ in range(k):
                    if not emit_next():
                        return

            # ----------------------------------------------------------
            # attention + normalization
            # ----------------------------------------------------------
            def attend_block(p, qb):
                jmax = 4 * qb + 4
                o_ps = [ops.tile([65, QB], F32, tag="o", name=f"o{hl}")
                        for hl in range(2)]
                for j0 in range(0, jmax, _JBATCH):
                    batch = range(j0, min(j0 + _JBATCH, jmax))
                    s_tiles = {}
                    e_tiles = {}
                    for j in batch:
                        qs = max(0, 128 * (j - 4 * qb))
                        s_t = sps.tile([128, 2 * QB], F32, tag="s",
                                       name=f"s{j}")
                        s_tiles[j] = s_t
                        for hl in range(2):
                            pb = 64 * hl
                            nc.tensor.matmul(
                                s_t[:, hl * QB + qs:(hl + 1) * QB],
                                lhsT=kT_sb[pb:pb + 64, p,
                                           j * 128:(j + 1) * 128],
                                rhs=qT_sb[pb:pb + 64, p,
                                          qb * QB + qs:(qb + 1) * QB],
                                start=True, stop=True)
                    for j in batch:
                        qs = max(0, 128 * (j - 4 * qb))
                        e_t = epool.tile([128, 2, QB], BF16, tag="e",
                                         name=f"e{j}")
                        e_tiles[j] = e_t
                        s_v = s_tiles[j].rearrange("q (h n) -> q h n", h=2)
                        nc.scalar.activation(e_t[:, :, qs:QB],
                                             s_v[:, :, qs:QB],
                                             AF.Exp, scale=SCALE)
                        if j >= 4 * qb:
                            for hl in range(2):
                                nc.vector.tensor_tensor(
                                    e_t[:, hl, qs:qs + 128],
                                    e_t[:, hl, qs:qs + 128],
                                    mask_sb[:], OP.mult)
                    feeder(1)
                    for j in batch:
                        qs = max(0, 128 * (j - 4 * qb))
                        e_t = e_tiles[j]
                        last = (j == jmax - 1)
                        for hl in range(2):
                            if j >= 4 * qb and _AV_SPLIT and j > 0:
                                if qs + 128 < QB:
                                    nc.tensor.matmul(
                                        o_ps[hl][:, qs + 128:QB],
                                        lhsT=v_tiles[p][:, j, hl, :],
                                        rhs=e_t[:, hl, qs + 128:QB],
                                        start=(j == 0), stop=False,
                                        skip_group_check=True)
                                nc.tensor.matmul(
                                    o_ps[hl][:, qs:qs + 128],
                                    lhsT=v_tiles[p][:, j, hl, :],
                                    rhs=e_t[:, hl, qs:qs + 128],
                                    start=(j == 0), stop=last,
                                    skip_group_check=True)
                            else:
                                nc.tensor.matmul(
                                    o_ps[hl][:, qs:QB],
                                    lhsT=v_tiles[p][:, j, hl, :],
                                    rhs=e_t[:, hl, qs:QB],
                                    start=(j == 0), stop=last,
                                    skip_group_check=True)
                    feeder(1)
                return o_ps

            def finish_pair_qb(p, qb, o_ps):
                """Evict + normalize both heads of the pair for this query
                block.  Denominators of both heads ride ONE DRAM bounce:
                write [2, 512], reload spread as [64, 16], reciprocal (16
                cols -> fast), write back, one broadcast load for both
                partition halves.  DMAs go on the gpsimd queue to keep the
                saturated sync queue clear."""
                row = p * NQB + qb
                stmp = npool.tile([1, 2 * QB], F32, tag="st", name="stmp")
                for hl in range(2):
                    nc.vector.tensor_copy(stmp[0:1, hl * QB:(hl + 1) * QB],
                                          o_ps[hl][64:65, :])
                nc.gpsimd.dma_start(snum[row:row + 1, :], stmp[:])
                st64 = npool.tile([64, 2 * QB // 64], F32, tag="sp",
                                  name="st64")
                nc.gpsimd.dma_start(
                    st64[:], snum[row].rearrange("(a b) -> a b", a=64))
                nc.vector.reciprocal(st64[:], st64[:])
                nc.gpsimd.dma_start(
                    srecd[row].rearrange("(a b) -> a b", a=64), st64[:])
                bcr = npool.tile([128, QB], F32, tag="bcr", name="bcr")
                for hl in range(2):
                    pb = 64 * hl
                    nc.gpsimd.dma_start(
                        bcr[pb:pb + 64, :],
                        srecd[row][None, hl * QB:(hl + 1) * QB]
                        .broadcast_to([64, QB]))
                    dst = o_keep[pb:pb + 64, p, qb * QB:(qb + 1) * QB]
                    nc.vector.tensor_copy(dst, o_ps[hl][0:64, :])
                    nc.vector.tensor_tensor(dst, dst, bcr[pb:pb + 64, :],
                                            OP.mult)

            # ----------------------------------------------------------
            # projection chunk c = y rows [256c, +256) + [1024+256c, +256):
            # needs o_keep query blocks qb=c//2 and qb=2+c//2 of ALL pairs,
            # so chunks 0,1 unlock after (pair3, qb2) and 2,3 after (3,3).
            # Emitted through the chain so they fill pair-3 attention idle.
            # ----------------------------------------------------------
            def proj_chunk_entries(c):
                entries = []
                for half in range(2):
                    for mm in range(2):
                        m = 2 * c + mm + 8 * half

                        def tile_work(m=m, c=c, half=half, mm=mm):
                            y_sb = ypool.tile([128, D], BF16, tag="y",
                                              name="y_sb")
                            for nch in range(2):
                                sl = slice(nch * 512, (nch + 1) * 512)
                                yp = qkps.tile([128, QB], F32, tag="pt",
                                               name="yp")
                                for kk in range(NP):
                                    nc.tensor.matmul(
                                        yp[:],
                                        lhsT=o_keep[:, kk,
                                                    m * 128:(m + 1) * 128],
                                        rhs=wp_sb[:, kk, sl],
                                        start=(kk == 0), stop=(kk == NP - 1))
                                nc.vector.tensor_tensor(
                                    y_sb[:, sl], yp[:], bias_bc[:, sl],
                                    OP.add)
                            nc.sync.dma_start(
                                y_part[c, half, mm * 128:(mm + 1) * 128],
                                y_sb[:])
                        entries.append(tile_work)

                def rs_out(c=c):
                    nc.gpsimd.collective_compute(
                        "ReduceScatter", OP.add,
                        replica_groups=[[0, 1], [2, 3], [4, 5], [6, 7]],
                        ins=[y_part[c]],
                        outs=[yred[c * 256:(c + 1) * 256, :]],
                    )
                    nc.sync.dma_start(yout[c * 256:(c + 1) * 256, :],
                                      yred[c * 256:(c + 1) * 256, :])
                entries.append(rs_out)
                return entries

            drain_until(0, 0)
            for p in range(NP):
                # pair 3 runs its query blocks as (0, 2, 1, 3) so projection
                # chunks 0,1 (need qb 0+2) unlock halfway through and their
                # ReduceScatters hide under the qb 1+3 attention
                qb_order = (0, 2, 1, 3) if p == NP - 1 else range(NQB)
                for qb in qb_order:
                    drain_until(p, qb)
                    o_ps = attend_block(p, qb)
                    finish_pair_qb(p, qb, o_ps)
                    if p == NP - 1 and qb in (2, 3):
                        for c in ((0, 1) if qb == 2 else (2, 3)):
                            for entry in proj_chunk_entries(c):
                                chain.append((entry, None))
            # drain remaining background work (late projection chunks)
            while emit_next():
                pass

        if _DEBUG_DUMP:
            nc.sync.dma_start(dbg[:], o_keep[:])

    nc.compile()
    return nc


# ---------------------------------------------------------------------------
# host-side sharding + entry point
# ---------------------------------------------------------------------------

_NC_CACHE = {}


def _get_nc():
    if "nc" not in _NC_CACHE:
        _NC_CACHE["nc"] = build_nc()
    return _NC_CACHE["nc"]


def _make_in_maps(x, Wq, Wk, Wv, Wp, bp):
    x = np.asarray(x, dtype=np.float32)
    Wq = np.asarray(Wq, dtype=np.float32)
    Wk = np.asarray(Wk, dtype=np.float32)
    Wv = np.asarray(Wv, dtype=np.float32)
    Wp = np.asarray(Wp, dtype=np.float32)
    bp = np.asarray(bp, dtype=np.float32)

    bf = mybir.dt.np(BF16)
    mask = np.zeros((128, 128), dtype=np.float32)
    k_idx = np.arange(128)[:, None]
    q_idx = np.arange(128)[None, :]
    mask[q_idx >= k_idx] = 1.0
    mask = mask.astype(bf)

    xTs = [np.ascontiguousarray(x[b].T).astype(bf) for b in range(B)]
    WpT = np.ascontiguousarray(Wp.T)
    in_maps = []
    for c in range(N_CORES):
        b, g = c // 2, c % 2
        rows = slice(512 * g, 512 * (g + 1))
        m = {
            "xT": xTs[b],
            "wqT": np.ascontiguousarray(Wq[rows, :].T).astype(bf),
            "wkT": np.ascontiguousarray(Wk[rows, :].T).astype(bf),
            "wvT": np.ascontiguousarray(Wv[rows, :].T).astype(bf),
            "wpT": np.ascontiguousarray(WpT[rows, :]).astype(bf),
            "bias": (bp if g == 0 else np.zeros_like(bp)).reshape(1, D),
            "mask": mask,
            "vone": np.ones((128, 32), dtype=bf),
        }
        in_maps.append(m)
    return in_maps


def kernel(x, Wq, Wk, Wv, Wp, bp, _trace=False, _mode=None):
    nc = _get_nc()
    in_maps = _make_in_maps(x, Wq, Wk, Wv, Wp, bp)
    res = _run_spmd(nc, in_maps, trace=_trace)
    out = np.empty((B, T, D), dtype=np.float32)
    for b in range(B):
        out[b, 0:QH] = res.results[2 * b]["yout"].astype(np.float32)
        out[b, QH:T] = res.results[2 * b + 1]["yout"].astype(np.float32)
    if _trace:
        kernel.last_results = res
    return out

